# revision 33
# baseline (speedup 1.0000x reference)
"""AttentionalPooler Trainium2 kernel (8-core data-parallel over batch).

Host precompute (parameter-only folding, numpy):
  - Qs = LN(query) @ Wq.T + bq, scaled by HD^-0.5, transposed -> QsT [D, NQ]
  - WkT/WvT: LN weight folded in, transposed -> [C, D]
  - projection biases folded: g_k dropped (softmax shift invariance),
    g_v folded into output bias bo2 = bo + Wo @ (Wv @ ln_k_b + bv)
Device per (core, local batch b):
  - LN(x) in natural [s, c] layout (bn_stats/bn_aggr + fused (x-m)*r, bf16 out)
  - kv -> kvT via DMA xbar transpose (bf16)
  - K^T = WkT.T @ kvT   [d, s];  V = kvT.T @ WvT  [s, d]   (PE, bf16)
  - scores[q, s] = QsT_h.T @ KT_h ; P = exp(scores) with accum_out -> Z
  - P normalized by 1/Z (per-partition), DMA-transposed -> PT [s, q]
  - ctxT_h [hd, q] = V_h.T @ PT ; out[q, d] = sum_h ctxT_h.T @ WoT_h (+ bo2)
"""

import os
import sys

import numpy as np

if "/opt/trn_rl_repo" not in sys.path:
    sys.path.insert(0, "/opt/trn_rl_repo")

import ml_dtypes

import concourse.bass as bass
import concourse.tile as tile
from concourse import bacc, mybir
from concourse.bass_utils import run_bass_kernel_spmd

D, C, H, NQ, B, S, HD, EPS = 1024, 1408, 8, 256, 16, 1024, 128, 1e-5
NCORES = 8
BL = B // NCORES          # local batches per core
CT = C // 128             # 11 c-tiles
DT = D // 128             # 8 d-tiles
ST = S // 128             # 8 s-tiles
QT = NQ // 128            # 2 q-tiles

F32 = mybir.dt.float32
BF16 = mybir.dt.bfloat16
AF = mybir.ActivationFunctionType
ALU = mybir.AluOpType
ts = bass.ts


def _build(nc, use_mask, use_bias):
    xd = nc.dram_tensor("x", [BL, S, C], F32, kind="ExternalInput").ap()
    qst = nc.dram_tensor("QsT", [D, NQ], BF16, kind="ExternalInput").ap()
    wkt = nc.dram_tensor("WkT", [C, D], BF16, kind="ExternalInput").ap()
    wvt = nc.dram_tensor("WvT", [C, D], BF16, kind="ExternalInput").ap()
    wot = nc.dram_tensor("WoT", [D, D], BF16, kind="ExternalInput").ap()
    outd = nc.dram_tensor("out", [BL, NQ, D], F32, kind="ExternalOutput").ap()
    idd = nc.dram_tensor("ident", [128, 128], BF16, kind="ExternalInput").ap()
    lmd = oned = bod = None
    if use_mask:
        # [128, BL*ST]: lm[p, b*ST+sc] = logmask[b, sc*128+p]
        lmd = nc.dram_tensor("logmask", [128, BL * ST], F32,
                             kind="ExternalInput").ap()
    if use_bias:
        bod = nc.dram_tensor("bo2", [1, D], BF16, kind="ExternalInput").ap()
        oned = nc.dram_tensor("onesrow", [1, 128], BF16,
                              kind="ExternalInput").ap()

    with tile.TileContext(nc) as tc:
        with (
            tc.tile_pool(name="tok", bufs=4) as tokp,
            tc.tile_pool(name="resident", bufs=1) as rp,
            tc.tile_pool(name="xin", bufs=2) as xp,
            tc.tile_pool(name="stats", bufs=6) as sp,
            tc.tile_pool(name="kv", bufs=1) as kvp,
            tc.tile_pool(name="kvt", bufs=2) as kvtp,
            tc.tile_pool(name="ktv", bufs=1) as ktvp,
            tc.tile_pool(name="pp", bufs=2) as pp,
            tc.tile_pool(name="ptp", bufs=2) as ptp,
            tc.tile_pool(name="ctx", bufs=1) as cxp,
            tc.tile_pool(name="outp", bufs=2) as op_,
            tc.tile_pool(name="psbig", bufs=2, space="PSUM") as psb,
            tc.tile_pool(name="pssmall", bufs=2, space="PSUM") as pss,
        ):
            wk_sb = rp.tile([128, CT, D], BF16, tag="wk")
            nc.gpsimd.dma_start(wk_sb[:], wkt.rearrange("(j p) d -> p j d", p=128))
            wv_sb = rp.tile([128, CT, D], BF16, tag="wv")
            nc.gpsimd.dma_start(wv_sb[:], wvt.rearrange("(j p) d -> p j d", p=128))
            wo_sb = rp.tile([128, DT, D], BF16, tag="wo")
            nc.gpsimd.dma_start(wo_sb[:], wot.rearrange("(h p) d -> p h d", p=128))
            qs_sb = rp.tile([128, H, NQ], BF16, tag="qs")
            nc.gpsimd.dma_start(qs_sb[:], qst.rearrange("(h p) q -> p h q", p=128))
            eps_sb = rp.tile([128, 1], F32, tag="eps")
            nc.vector.memset(eps_sb[:], EPS)
            onec_sb = rp.tile([128, 1], BF16, tag="onec")
            nc.vector.memset(onec_sb[:], 1.0)
            id_sb = rp.tile([128, 128], BF16, tag="ident")
            nc.gpsimd.dma_start(id_sb[:], idd)
            lm_sb = on_sb = bo_sb = None
            if use_mask:
                lm_sb = rp.tile([128, BL * ST], F32, tag="lm")
                nc.gpsimd.dma_start(lm_sb[:], lmd)
            if use_bias:
                bo_sb = rp.tile([1, D], BF16, tag="bo")
                nc.gpsimd.dma_start(bo_sb[:], bod)
                on_sb = rp.tile([1, 128], BF16, tag="ones")
                nc.gpsimd.dma_start(on_sb[:], oned)

            for b in range(BL):
                # ---- LayerNorm + transpose ----
                kvt_sb = kvtp.tile([128, CT, S], BF16, tag="kvt")
                kvs = []
                for t in range(ST):
                    kv_sb = kvp.tile([128, C], BF16, tag=f"kv{t}")
                    kvs.append(kv_sb)
                    xt = xp.tile([128, C], F32, tag="x")
                    nc.gpsimd.dma_start(xt[:], xd[b, ts(t, 128), :])
                    st = sp.tile([128, 4, 6], F32, tag="bns")
                    xv = xt[:].rearrange("p (a f) -> p a f", a=4)
                    for a in range(4):
                        nc.vector.bn_stats(st[:, a, :], xv[:, a, :])
                    mv = sp.tile([128, 2], F32, tag="mv")
                    nc.vector.bn_aggr(mv[:], st[:])
                    sq = sp.tile([128, 1], F32, tag="sq")
                    nc.scalar.activation(sq[:], mv[:, 1:2], AF.Sqrt, bias=eps_sb[:])
                    r = sp.tile([128, 1], F32, tag="r")
                    nc.vector.reciprocal(r[:], sq[:])
                    nc.vector.tensor_scalar(
                        kv_sb[:], xt[:], mv[:, 0:1], r[:],
                        ALU.subtract, ALU.mult,
                    )
                for j in range(CT):
                    pst = pss.tile([128, 1024], BF16, tag="pst")
                    for t in range(ST):
                        nc.tensor.transpose(
                            pst[:, ts(t, 128)], kvs[t][:, ts(j, 128)], id_sb[:]
                        )
                    if j % 2 == 0:
                        nc.scalar.copy(kvt_sb[:, j, :], pst[:])
                    else:
                        nc.vector.tensor_copy(kvt_sb[:, j, :], pst[:])

                # ---- K^T and V projections ----
                kt_sb = ktvp.tile([128, DT, S], BF16, tag="kt")
                v_sb = ktvp.tile([128, ST, D], BF16, tag="v")
                for dt in range(DT):
                    ps = psb.tile([128, 1024], F32, tag="ps")
                    for j in range(CT):
                        for n in range(2):
                            nc.tensor.matmul(
                                ps[:, ts(n, 512)],
                                lhsT=wk_sb[:, j, ts(dt, 128)],
                                rhs=kvt_sb[:, j, ts(n, 512)],
                                start=(j == 0), stop=(j == CT - 1),
                            )
                    nc.scalar.copy(kt_sb[:, dt, :], ps[:])
                for t in range(ST):
                    ps = psb.tile([128, 1024], F32, tag="ps")
                    for j in range(CT):
                        for n in range(2):
                            nc.tensor.matmul(
                                ps[:, ts(n, 512)],
                                lhsT=kvt_sb[:, j, ts(t, 128)],
                                rhs=wv_sb[:, j, ts(n, 512)],
                                start=(j == 0), stop=(j == CT - 1),
                            )
                    nc.vector.tensor_copy(v_sb[:, t, :], ps[:])

                # ---- attention per head (scoresT [s, q] orientation) ----
                ctx_sb = cxp.tile([128, H, NQ], BF16, tag="ctx")
                for h in range(H):
                    # P[s, q] = exp(scoresT); two groups of 4 s-chunks
                    p_sb = pp.tile([128, ST, NQ], BF16, tag="p")
                    for g in range(2):
                        ps = psb.tile([128, 4, NQ], F32, tag="ps")
                        for i in range(4):
                            sc = g * 4 + i
                            nc.tensor.matmul(
                                ps[:, i, :],
                                lhsT=kt_sb[:, h, ts(sc, 128)],
                                rhs=qs_sb[:, h, :],
                                start=True, stop=True,
                            )
                        if use_mask:
                            for i in range(4):
                                sc = g * 4 + i
                                nc.scalar.activation(
                                    p_sb[:, sc, :], ps[:, i, :], AF.Exp,
                                    bias=lm_sb[:, b * ST + sc:b * ST + sc + 1],
                                )
                        else:
                            nc.scalar.activation(
                                p_sb[:, ts(g, 4), :].rearrange("p a q -> p (a q)"),
                                ps[:].rearrange("p a q -> p (a q)"), AF.Exp,
                            )
                    # Z[1, q] = sum_s P  (ones-column matmuls)
                    psz = pss.tile([1, NQ], F32, tag="psc")
                    for sc in range(ST):
                        nc.tensor.matmul(
                            psz[:],
                            lhsT=onec_sb[:],
                            rhs=p_sb[:, sc, :],
                            start=(sc == 0), stop=(sc == ST - 1),
                        )
                    zi = sp.tile([1, NQ], F32, tag="zi")
                    nc.vector.reciprocal(zi[:], psz[:])
                    zb = ptp.tile([128, NQ], F32, tag="zb")
                    nc.gpsimd.partition_broadcast(zb[:], zi[:])
                    # ctxT_h [hd, q], normalized by broadcast 1/Z
                    psc = pss.tile([128, NQ], F32, tag="psc")
                    for sc in range(ST):
                        nc.tensor.matmul(
                            psc[:],
                            lhsT=v_sb[:, sc, ts(h, 128)],
                            rhs=p_sb[:, sc, :],
                            start=(sc == 0), stop=(sc == ST - 1),
                        )
                    nc.vector.tensor_tensor(
                        ctx_sb[:, h, :], psc[:], zb[:], ALU.mult,
                    )

                # ---- output projection ----
                for qt in range(QT):
                    ps = psb.tile([128, 1024], F32, tag="ps")
                    for h in range(H):
                        for n in range(2):
                            nc.tensor.matmul(
                                ps[:, ts(n, 512)],
                                lhsT=ctx_sb[:, h, ts(qt, 128)],
                                rhs=wo_sb[:, h, ts(n, 512)],
                                start=(h == 0),
                                stop=(h == H - 1) and not use_bias,
                            )
                            if use_bias:
                                nc.tensor.matmul(
                                    ps[:, ts(n, 512)],
                                    lhsT=on_sb[:, :],
                                    rhs=bo_sb[:, ts(n, 512)],
                                    start=False, stop=True,
                                )
                    ot = op_.tile([128, 1024], F32, tag="ot")
                    if qt % 2 == 0:
                        nc.vector.tensor_copy(ot[:], ps[:])
                    else:
                        nc.scalar.copy(ot[:], ps[:])
                    nc.gpsimd.dma_start(outd[b, ts(qt, 128), :], ot[:])
    return nc


def kernel(x, attn_mask, query, ln_q_w, ln_q_b, ln_k_w, ln_k_b,
           Wq, Wk, Wv, bq, bk, bv, Wo, bo):
    x = np.asarray(x)
    attn_mask = np.asarray(attn_mask)
    # ---- host precompute (parameter folding, fp64) ----
    q64 = np.asarray(query).astype(np.float64)
    m = q64.mean(-1, keepdims=True)
    v = ((q64 - m) ** 2).mean(-1, keepdims=True)
    qln = ((q64 - m) / np.sqrt(v + EPS)) * np.asarray(ln_q_w) + np.asarray(ln_q_b)
    Qs = (qln @ np.asarray(Wq).T.astype(np.float64) + np.asarray(bq)) * (HD ** -0.5)
    QsT = np.ascontiguousarray(Qs.T).astype(ml_dtypes.bfloat16)
    WkT = np.ascontiguousarray((np.asarray(Wk) * np.asarray(ln_k_w)[None, :]).T
                               ).astype(ml_dtypes.bfloat16)
    WvT = np.ascontiguousarray((np.asarray(Wv) * np.asarray(ln_k_w)[None, :]).T
                               ).astype(ml_dtypes.bfloat16)
    WoT = np.ascontiguousarray(np.asarray(Wo).T).astype(ml_dtypes.bfloat16)
    g_v = np.asarray(Wv).astype(np.float64) @ np.asarray(ln_k_b) + np.asarray(bv)
    bo2 = (np.asarray(bo) + np.asarray(Wo).astype(np.float64) @ g_v)

    use_mask = not attn_mask.all()
    use_bias = bool(np.abs(bo2).max() > 0)

    nc = bacc.Bacc("TRN2", target_bir_lowering=False, debug=False,
                   num_devices=NCORES)
    _build(nc, use_mask, use_bias)
    nc.finalize()

    in_maps = []
    for c in range(NCORES):
        im = {
            "x": np.ascontiguousarray(x[c * BL:(c + 1) * BL]),
            "QsT": QsT, "WkT": WkT, "WvT": WvT, "WoT": WoT,
            "ident": np.eye(128, dtype=ml_dtypes.bfloat16),
        }
        if use_mask:
            lm = np.where(attn_mask[c * BL:(c + 1) * BL], 0.0, -60.0)
            im["logmask"] = np.ascontiguousarray(
                lm.reshape(BL, ST, 128).transpose(2, 0, 1).reshape(128, BL * ST)
            ).astype(np.float32)
        if use_bias:
            im["bo2"] = bo2.reshape(1, D).astype(ml_dtypes.bfloat16)
        if use_mask or use_bias:
            im["onesrow"] = np.ones((1, 128), ml_dtypes.bfloat16)
        in_maps.append(im)

    res = run_bass_kernel_spmd(
        nc, in_maps, core_ids=list(range(NCORES)), trace=False,
    )
    out = np.concatenate([r["out"] for r in res.results], axis=0)

    if os.environ.get("KTRACE", "0") == "1":
        _time_exec(nc, in_maps)

    return out.astype(np.float32)


def _time_exec(nc, in_maps, iters=20):
    """Wall-clock the compiled NEFF execute with device-resident inputs."""
    import time

    import jax
    from jax.sharding import Mesh, NamedSharding, PartitionSpec
    from jax.experimental.shard_map import shard_map
    from concourse import bass2jax, mybir as mb

    bass2jax.install_neuronx_cc_hook()
    partition_name = (nc.partition_id_tensor.name
                      if nc.partition_id_tensor else None)
    in_names, out_names, out_avals, zero_outs = [], [], [], []
    for alloc in nc.m.functions[0].allocations:
        if not isinstance(alloc, mb.MemoryLocationSet):
            continue
        name = alloc.memorylocations[0].name
        if alloc.kind == "ExternalInput" and name != partition_name:
            in_names.append(name)
        elif alloc.kind == "ExternalOutput":
            shape = tuple(alloc.tensor_shape)
            dtype = mb.dt.np(alloc.dtype)
            out_names.append(name)
            out_avals.append(jax.core.ShapedArray(shape, dtype))
            zero_outs.append(np.zeros(shape, dtype))
    n_params = len(in_names)
    all_names = in_names + out_names

    def _body(*args):
        operands = list(args)
        if partition_name is not None:
            operands.append(bass2jax.partition_id_tensor())
        return tuple(bass2jax._bass_exec_p.bind(
            *operands, out_avals=tuple(out_avals),
            in_names=tuple(all_names + ([partition_name] if partition_name else [])),
            out_names=tuple(out_names),
            lowering_input_output_aliases=(),
            sim_require_finite=True, sim_require_nnan=True, nc=nc,
        ))

    devices = jax.devices()[:NCORES]
    mesh = Mesh(np.asarray(devices), ("core",))
    nsh = NamedSharding(mesh, PartitionSpec("core"))
    f = jax.jit(shard_map(
        _body, mesh=mesh,
        in_specs=(PartitionSpec("core"),) * (n_params + len(out_names)),
        out_specs=(PartitionSpec("core"),) * len(out_names),
        check_rep=False,
    ), keep_unused=True)
    concat_in = [
        jax.device_put(
            np.concatenate([np.asarray(in_maps[c][n]) for c in range(NCORES)], 0),
            nsh)
        for n in in_names
    ]
    concat_zero = [
        jax.device_put(np.zeros((NCORES * z.shape[0], *z.shape[1:]), z.dtype), nsh)
        for z in zero_outs
    ]
    r = f(*concat_in, *concat_zero)
    jax.block_until_ready(r)
    times = []
    for _ in range(iters):
        t0 = time.perf_counter()
        r = f(*concat_in, *concat_zero)
        jax.block_until_ready(r)
        times.append(time.perf_counter() - t0)
    best = min(times)
    print(f"exec wall best/median over {iters}: "
          f"{best * 1e6:.1f} / {sorted(times)[len(times) // 2] * 1e6:.1f} us")
    print(f"HW exec time: {int(best * 1e9)} ns")


if __name__ == "__main__":
    rng = np.random.default_rng(0)
    ins = {
        "x": rng.standard_normal((B, S, C), dtype=np.float32),
        "attn_mask": np.ones((B, S), bool),
        "query": rng.standard_normal((NQ, D), dtype=np.float32),
        "ln_q_w": np.ones(D, np.float32), "ln_q_b": np.zeros(D, np.float32),
        "ln_k_w": np.ones(C, np.float32), "ln_k_b": np.zeros(C, np.float32),
        "Wq": (rng.standard_normal((D, D)) * 0.02).astype(np.float32),
        "Wk": (rng.standard_normal((D, C)) * 0.02).astype(np.float32),
        "Wv": (rng.standard_normal((D, C)) * 0.02).astype(np.float32),
        "bq": np.zeros(D, np.float32), "bk": np.zeros(D, np.float32),
        "bv": np.zeros(D, np.float32),
        "Wo": (rng.standard_normal((D, D)) * 0.02).astype(np.float32),
        "bo": np.zeros(D, np.float32),
    }
    out = kernel(**ins)
    print("out", out.shape, out.dtype, np.abs(out).max())


# revision 41
# speedup vs baseline: 1.3134x; 1.3134x over previous
"""AttentionalPooler Trainium2 kernel (8-core data-parallel over batch).

Host precompute (parameter-only folding, numpy):
  - Qs = LN(query) @ Wq.T + bq, scaled by HD^-0.5, transposed -> QsT [D, NQ]
  - WkT/WvT: LN weight folded in, transposed -> [C, D]
  - projection biases folded: g_k dropped (softmax shift invariance),
    g_v folded into output bias bo2 = bo + Wo @ (Wv @ ln_k_b + bv)
Device per (core, local batch b):
  - LN(x) in natural [s, c] layout (bn_stats/bn_aggr + fused (x-m)*r, bf16 out)
  - kv -> kvT via DMA xbar transpose (bf16)
  - K^T = WkT.T @ kvT   [d, s];  V = kvT.T @ WvT  [s, d]   (PE, bf16)
  - scores[q, s] = QsT_h.T @ KT_h ; P = exp(scores) with accum_out -> Z
  - P normalized by 1/Z (per-partition), DMA-transposed -> PT [s, q]
  - ctxT_h [hd, q] = V_h.T @ PT ; out[q, d] = sum_h ctxT_h.T @ WoT_h (+ bo2)
"""

import os
import sys

import numpy as np

if "/opt/trn_rl_repo" not in sys.path:
    sys.path.insert(0, "/opt/trn_rl_repo")

import ml_dtypes

import concourse.bass as bass
import concourse.tile as tile
from concourse import bacc, mybir
from concourse.bass_utils import run_bass_kernel_spmd

D, C, H, NQ, B, S, HD, EPS = 1024, 1408, 8, 256, 16, 1024, 128, 1e-5
NCORES = 8
BL = B // NCORES          # local batches per core
CT = C // 128             # 11 c-tiles
DT = D // 128             # 8 d-tiles
ST = S // 128             # 8 s-tiles
QT = NQ // 128            # 2 q-tiles

F32 = mybir.dt.float32
BF16 = mybir.dt.bfloat16
AF = mybir.ActivationFunctionType
ALU = mybir.AluOpType
ts = bass.ts


def _build(nc, use_mask, use_bias):
    xd = nc.dram_tensor("x", [BL, S, C], F32, kind="ExternalInput").ap()
    qst = nc.dram_tensor("QsT", [D, NQ], BF16, kind="ExternalInput").ap()
    wkt = nc.dram_tensor("WkT", [C, D], BF16, kind="ExternalInput").ap()
    wvt = nc.dram_tensor("WvT", [C, D], BF16, kind="ExternalInput").ap()
    wot = nc.dram_tensor("WoT", [D, D], BF16, kind="ExternalInput").ap()
    outd = nc.dram_tensor("out", [BL, NQ, D], F32, kind="ExternalOutput").ap()
    idd = nc.dram_tensor("ident", [128, 128], BF16, kind="ExternalInput").ap()
    lmd = oned = bod = None
    if use_mask:
        lmd = nc.dram_tensor("logmask", [BL, S], BF16, kind="ExternalInput").ap()
    if use_bias:
        bod = nc.dram_tensor("bo2", [1, D], BF16, kind="ExternalInput").ap()
    if use_mask or use_bias:
        oned = nc.dram_tensor("onesrow", [1, 128], BF16,
                              kind="ExternalInput").ap()

    with tile.TileContext(nc) as tc:
        with (
            tc.tile_pool(name="tok", bufs=4) as tokp,
            tc.tile_pool(name="resident", bufs=1) as rp,
            tc.tile_pool(name="xin", bufs=2) as xp,
            tc.tile_pool(name="stats", bufs=6) as sp,
            tc.tile_pool(name="kv", bufs=1) as kvp,
            tc.tile_pool(name="kvt", bufs=2) as kvtp,
            tc.tile_pool(name="ktv", bufs=1) as ktvp,
            tc.tile_pool(name="pp", bufs=2) as pp,
            tc.tile_pool(name="ptp", bufs=2) as ptp,
            tc.tile_pool(name="ctx", bufs=1) as cxp,
            tc.tile_pool(name="outp", bufs=2) as op_,
            tc.tile_pool(name="psbig", bufs=2, space="PSUM") as psb,
            tc.tile_pool(name="pssmall", bufs=2, space="PSUM") as pss,
        ):
            wk_sb = rp.tile([128, CT, D], BF16, tag="wk")
            nc.gpsimd.dma_start(wk_sb[:], wkt.rearrange("(j p) d -> p j d", p=128))
            wv_sb = rp.tile([128, CT, D], BF16, tag="wv")
            nc.gpsimd.dma_start(wv_sb[:], wvt.rearrange("(j p) d -> p j d", p=128))
            wo_sb = rp.tile([128, DT, D], BF16, tag="wo")
            nc.gpsimd.dma_start(wo_sb[:], wot.rearrange("(h p) d -> p h d", p=128))
            qs_sb = rp.tile([128, H, NQ], BF16, tag="qs")
            nc.gpsimd.dma_start(qs_sb[:], qst.rearrange("(h p) q -> p h q", p=128))
            eps_sb = rp.tile([128, 1], F32, tag="eps")
            nc.vector.memset(eps_sb[:], EPS)
            onec_sb = rp.tile([128, 1], BF16, tag="onec")
            nc.vector.memset(onec_sb[:], 1.0)
            id_sb = rp.tile([128, 128], BF16, tag="ident")
            nc.gpsimd.dma_start(id_sb[:], idd)
            lm_sb = on_sb = bo_sb = None
            if use_mask:
                lm_sb = rp.tile([1, BL, S], BF16, tag="lm")
                nc.gpsimd.dma_start(lm_sb[:], lmd.rearrange("b s -> 1 b s"))
            if use_bias:
                bo_sb = rp.tile([1, D], BF16, tag="bo")
                nc.gpsimd.dma_start(bo_sb[:], bod)
            if use_mask or use_bias:
                on_sb = rp.tile([1, 128], BF16, tag="ones")
                nc.gpsimd.dma_start(on_sb[:], oned)

            for b in range(BL):
                # ---- LayerNorm + transpose ----
                kvt_sb = kvtp.tile([128, CT, S], BF16, tag="kvt")
                kvs = []
                for t in range(ST):
                    kv_sb = kvp.tile([128, C], BF16, tag=f"kv{t}")
                    kvs.append(kv_sb)
                    xt = xp.tile([128, C], F32, tag="x")
                    nc.gpsimd.dma_start(xt[:], xd[b, ts(t, 128), :])
                    st = sp.tile([128, 4, 6], F32, tag="bns")
                    xv = xt[:].rearrange("p (a f) -> p a f", a=4)
                    for a in range(4):
                        nc.vector.bn_stats(st[:, a, :], xv[:, a, :])
                    mv = sp.tile([128, 2], F32, tag="mv")
                    nc.vector.bn_aggr(mv[:], st[:])
                    sq = sp.tile([128, 1], F32, tag="sq")
                    nc.scalar.activation(sq[:], mv[:, 1:2], AF.Sqrt, bias=eps_sb[:])
                    r = sp.tile([128, 1], F32, tag="r")
                    nc.vector.reciprocal(r[:], sq[:])
                    nc.vector.tensor_scalar(
                        kv_sb[:], xt[:], mv[:, 0:1], r[:],
                        ALU.subtract, ALU.mult,
                    )
                for j in range(CT):
                    pst = pss.tile([128, 1024], BF16, tag="pst")
                    for t in range(ST):
                        nc.tensor.transpose(
                            pst[:, ts(t, 128)], kvs[t][:, ts(j, 128)], id_sb[:]
                        )
                    if j % 2 == 0:
                        nc.scalar.copy(kvt_sb[:, j, :], pst[:])
                    else:
                        nc.vector.tensor_copy(kvt_sb[:, j, :], pst[:])

                # ---- K^T and V projections ----
                kt_sb = ktvp.tile([128, DT, S], BF16, tag="kt")
                v_sb = ktvp.tile([128, ST, D], BF16, tag="v")
                for dt in range(DT):
                    ps = psb.tile([128, 1024], F32, tag="ps")
                    for j in range(CT):
                        for n in range(2):
                            nc.tensor.matmul(
                                ps[:, ts(n, 512)],
                                lhsT=wk_sb[:, j, ts(dt, 128)],
                                rhs=kvt_sb[:, j, ts(n, 512)],
                                start=(j == 0), stop=(j == CT - 1),
                            )
                    nc.scalar.copy(kt_sb[:, dt, :], ps[:])
                for t in range(ST):
                    ps = psb.tile([128, 1024], F32, tag="ps")
                    for j in range(CT):
                        for n in range(2):
                            nc.tensor.matmul(
                                ps[:, ts(n, 512)],
                                lhsT=kvt_sb[:, j, ts(t, 128)],
                                rhs=wv_sb[:, j, ts(n, 512)],
                                start=(j == 0), stop=(j == CT - 1),
                            )
                    nc.vector.tensor_copy(v_sb[:, t, :], ps[:])

                # ---- attention per head (natural scores, DMA-transposed P) ----
                ctx_sb = cxp.tile([128, H, NQ], BF16, tag="ctx")
                for h in range(H):
                    # scores[q, s] -> exp (Z via accum_out) -> normalize
                    pn_sb = pp.tile([128, QT, S], BF16, tag="pn")
                    z = sp.tile([128, QT], F32, tag="z")
                    zi = sp.tile([128, QT], F32, tag="zi")
                    for qt in range(QT):
                        ps = psb.tile([128, 1024], F32, tag="ps")
                        for n in range(2):
                            nc.tensor.matmul(
                                ps[:, ts(n, 512)],
                                lhsT=qs_sb[:, h, ts(qt, 128)],
                                rhs=kt_sb[:, h, ts(n, 512)],
                                start=True, stop=not use_mask,
                            )
                            if use_mask:
                                nc.tensor.matmul(
                                    ps[:, ts(n, 512)],
                                    lhsT=on_sb[:, :],
                                    rhs=lm_sb[:, b, ts(n, 512)],
                                    start=False, stop=True,
                                )
                        nc.scalar.activation(
                            pn_sb[:, qt, :], ps[:], AF.Exp,
                            accum_out=z[:, qt:qt + 1],
                        )
                    nc.vector.reciprocal(zi[:], z[:])
                    for qt in range(QT):
                        nc.vector.tensor_scalar_mul(
                            pn_sb[:, qt, :], pn_sb[:, qt, :], zi[:, qt:qt + 1]
                        )
                    pt_sb = ptp.tile([128, ST, NQ], BF16, tag="pt")
                    for qt in range(QT):
                        nc.sync.dma_start_transpose(
                            pt_sb[:, :, ts(qt, 128)], pn_sb[:, qt, :]
                        )
                    psc = pss.tile([128, NQ], F32, tag="psc")
                    for sc in range(ST):
                        nc.tensor.matmul(
                            psc[:],
                            lhsT=v_sb[:, sc, ts(h, 128)],
                            rhs=pt_sb[:, sc, :],
                            start=(sc == 0), stop=(sc == ST - 1),
                        )
                    if h % 2 == 0:
                        nc.vector.tensor_copy(ctx_sb[:, h, :], psc[:])
                    else:
                        nc.scalar.copy(ctx_sb[:, h, :], psc[:])

                # ---- output projection ----
                for qt in range(QT):
                    ps = psb.tile([128, 1024], F32, tag="ps")
                    for h in range(H):
                        for n in range(2):
                            nc.tensor.matmul(
                                ps[:, ts(n, 512)],
                                lhsT=ctx_sb[:, h, ts(qt, 128)],
                                rhs=wo_sb[:, h, ts(n, 512)],
                                start=(h == 0),
                                stop=(h == H - 1) and not use_bias,
                            )
                            if use_bias:
                                nc.tensor.matmul(
                                    ps[:, ts(n, 512)],
                                    lhsT=on_sb[:, :],
                                    rhs=bo_sb[:, ts(n, 512)],
                                    start=False, stop=True,
                                )
                    ot = op_.tile([128, 1024], F32, tag="ot")
                    if qt % 2 == 0:
                        nc.vector.tensor_copy(ot[:], ps[:])
                    else:
                        nc.scalar.copy(ot[:], ps[:])
                    nc.gpsimd.dma_start(outd[b, ts(qt, 128), :], ot[:])
    return nc


def kernel(x, attn_mask, query, ln_q_w, ln_q_b, ln_k_w, ln_k_b,
           Wq, Wk, Wv, bq, bk, bv, Wo, bo):
    x = np.asarray(x)
    attn_mask = np.asarray(attn_mask)
    # ---- host precompute (parameter folding, fp64) ----
    q64 = np.asarray(query).astype(np.float64)
    m = q64.mean(-1, keepdims=True)
    v = ((q64 - m) ** 2).mean(-1, keepdims=True)
    qln = ((q64 - m) / np.sqrt(v + EPS)) * np.asarray(ln_q_w) + np.asarray(ln_q_b)
    Qs = (qln @ np.asarray(Wq).T.astype(np.float64) + np.asarray(bq)) * (HD ** -0.5)
    QsT = np.ascontiguousarray(Qs.T).astype(ml_dtypes.bfloat16)
    WkT = np.ascontiguousarray((np.asarray(Wk) * np.asarray(ln_k_w)[None, :]).T
                               ).astype(ml_dtypes.bfloat16)
    WvT = np.ascontiguousarray((np.asarray(Wv) * np.asarray(ln_k_w)[None, :]).T
                               ).astype(ml_dtypes.bfloat16)
    WoT = np.ascontiguousarray(np.asarray(Wo).T).astype(ml_dtypes.bfloat16)
    g_v = np.asarray(Wv).astype(np.float64) @ np.asarray(ln_k_b) + np.asarray(bv)
    bo2 = (np.asarray(bo) + np.asarray(Wo).astype(np.float64) @ g_v)

    use_mask = not attn_mask.all()
    use_bias = bool(np.abs(bo2).max() > 0)

    nc = bacc.Bacc("TRN2", target_bir_lowering=False, debug=False,
                   num_devices=NCORES)
    _build(nc, use_mask, use_bias)
    nc.finalize()

    in_maps = []
    for c in range(NCORES):
        im = {
            "x": np.ascontiguousarray(x[c * BL:(c + 1) * BL]),
            "QsT": QsT, "WkT": WkT, "WvT": WvT, "WoT": WoT,
            "ident": np.eye(128, dtype=ml_dtypes.bfloat16),
        }
        if use_mask:
            im["logmask"] = np.where(
                attn_mask[c * BL:(c + 1) * BL], 0.0, -60.0
            ).astype(ml_dtypes.bfloat16)
        if use_bias:
            im["bo2"] = bo2.reshape(1, D).astype(ml_dtypes.bfloat16)
        if use_mask or use_bias:
            im["onesrow"] = np.ones((1, 128), ml_dtypes.bfloat16)
        in_maps.append(im)

    res = run_bass_kernel_spmd(
        nc, in_maps, core_ids=list(range(NCORES)), trace=False,
    )
    out = np.concatenate([r["out"] for r in res.results], axis=0)

    if os.environ.get("KTRACE", "0") == "1":
        _time_exec(nc, in_maps)

    return out.astype(np.float32)


def _time_exec(nc, in_maps, iters=20):
    """Wall-clock the compiled NEFF execute with device-resident inputs."""
    import time

    import jax
    from jax.sharding import Mesh, NamedSharding, PartitionSpec
    from jax.experimental.shard_map import shard_map
    from concourse import bass2jax, mybir as mb

    bass2jax.install_neuronx_cc_hook()
    partition_name = (nc.partition_id_tensor.name
                      if nc.partition_id_tensor else None)
    in_names, out_names, out_avals, zero_outs = [], [], [], []
    for alloc in nc.m.functions[0].allocations:
        if not isinstance(alloc, mb.MemoryLocationSet):
            continue
        name = alloc.memorylocations[0].name
        if alloc.kind == "ExternalInput" and name != partition_name:
            in_names.append(name)
        elif alloc.kind == "ExternalOutput":
            shape = tuple(alloc.tensor_shape)
            dtype = mb.dt.np(alloc.dtype)
            out_names.append(name)
            out_avals.append(jax.core.ShapedArray(shape, dtype))
            zero_outs.append(np.zeros(shape, dtype))
    n_params = len(in_names)
    all_names = in_names + out_names

    def _body(*args):
        operands = list(args)
        if partition_name is not None:
            operands.append(bass2jax.partition_id_tensor())
        return tuple(bass2jax._bass_exec_p.bind(
            *operands, out_avals=tuple(out_avals),
            in_names=tuple(all_names + ([partition_name] if partition_name else [])),
            out_names=tuple(out_names),
            lowering_input_output_aliases=(),
            sim_require_finite=True, sim_require_nnan=True, nc=nc,
        ))

    devices = jax.devices()[:NCORES]
    mesh = Mesh(np.asarray(devices), ("core",))
    nsh = NamedSharding(mesh, PartitionSpec("core"))
    f = jax.jit(shard_map(
        _body, mesh=mesh,
        in_specs=(PartitionSpec("core"),) * (n_params + len(out_names)),
        out_specs=(PartitionSpec("core"),) * len(out_names),
        check_rep=False,
    ), keep_unused=True)
    concat_in = [
        jax.device_put(
            np.concatenate([np.asarray(in_maps[c][n]) for c in range(NCORES)], 0),
            nsh)
        for n in in_names
    ]
    concat_zero = [
        jax.device_put(np.zeros((NCORES * z.shape[0], *z.shape[1:]), z.dtype), nsh)
        for z in zero_outs
    ]
    r = f(*concat_in, *concat_zero)
    jax.block_until_ready(r)
    times = []
    for _ in range(iters):
        t0 = time.perf_counter()
        r = f(*concat_in, *concat_zero)
        jax.block_until_ready(r)
        times.append(time.perf_counter() - t0)
    best = min(times)
    print(f"exec wall best/median over {iters}: "
          f"{best * 1e6:.1f} / {sorted(times)[len(times) // 2] * 1e6:.1f} us")
    print(f"HW exec time: {int(best * 1e9)} ns")


if __name__ == "__main__":
    rng = np.random.default_rng(0)
    ins = {
        "x": rng.standard_normal((B, S, C), dtype=np.float32),
        "attn_mask": np.ones((B, S), bool),
        "query": rng.standard_normal((NQ, D), dtype=np.float32),
        "ln_q_w": np.ones(D, np.float32), "ln_q_b": np.zeros(D, np.float32),
        "ln_k_w": np.ones(C, np.float32), "ln_k_b": np.zeros(C, np.float32),
        "Wq": (rng.standard_normal((D, D)) * 0.02).astype(np.float32),
        "Wk": (rng.standard_normal((D, C)) * 0.02).astype(np.float32),
        "Wv": (rng.standard_normal((D, C)) * 0.02).astype(np.float32),
        "bq": np.zeros(D, np.float32), "bk": np.zeros(D, np.float32),
        "bv": np.zeros(D, np.float32),
        "Wo": (rng.standard_normal((D, D)) * 0.02).astype(np.float32),
        "bo": np.zeros(D, np.float32),
    }
    out = kernel(**ins)
    print("out", out.shape, out.dtype, np.abs(out).max())
